# revision 31
# baseline (speedup 1.0000x reference)
"""Trainium2 Bass kernel for a leaky-integrate-fire (LIF) scan.

Reference computation (forward values only):
    v_t   = mem_{t-1} * 0.25 + x_t          (mem_0 carry = 0)
    s_t   = (v_t > 1.0) ? 1.0 : 0.0         (spike, the output)
    mem_t = (v_t <= 1.0) * v_t              (hard reset)

x: [T=32, B=64, N=16384] f32. Elementwise over (B, N), sequential over T.
Sharding: data-parallel over flattened B*N across 8 cores. Each core's slab
is laid out [P=128, T=32, F=1024] in DRAM.

Design: the F=1024 columns are split [E | A | W] across three chain engines
so no single engine owns the serial 2-op/step recurrence:
- E cols: PE computes v = 1*x + 0.25*mem as two fp32 diagonal matmuls into
  PSUM (bitwise-exact: diag weights are powers of two, verified on hw), DVE
  does only the reset (v<=1)*v back to SBUF.
- A cols: classic DVE chain (2 scalar_tensor_tensor per step).
- W cols: gpsimd (Pool) chain with tensor_scalar/tensor_tensor.
Spikes: ACT computes Sign(v-1) in {-1,0,+1} as fp16 for all F columns (exact:
v-1 is Sterbenz-exact in [0.5,2), and sign(v-1)==1 <=> v>1 exactly). PE then
packs 8 steps of signs into one PSUM word via diag(4^j) fp16 matmuls
(word = sum_j sign_j * 4^j, a balanced base-4 integer, |word| <= 21845),
ACT converts the word to int16, and the host decodes digits. This cuts
store traffic 4x vs int8-per-step and keeps the output bit-exact.

All on-device arithmetic reproduces the jax f32 reference bitwise; the
host decode maps digit +1 -> spike, {0,-1} -> no spike (v==1 gives digit 0,
matching the reference's strict v>1).
"""

import numpy as np

T = 32
B = 64
N = 16384
NCORES = 8
P = 128                      # SBUF partitions
F = (B // NCORES) * N // P   # 1024 free-dim columns per step per core
TB = 4                       # timesteps per DMA block (2 MiB loads)
WPACK = 8                    # steps packed per output word
NW = T // WPACK              # output words per column
E = 360                      # columns whose affine runs on PE
A = 480                      # columns on the DVE chain
W = F - E - A                # columns on the gpsimd chain
DECAY = 0.25
VTH = 1.0

_CACHE = {}


def _build_program():
    import concourse.bacc as bacc
    import concourse.tile as tile
    from concourse import mybir

    nc = bacc.Bacc(
        target_bir_lowering=False,
        debug=False,
        enable_asserts=False,
        num_devices=NCORES,
    )
    f32 = mybir.dt.float32
    f16 = mybir.dt.float16
    i16 = mybir.dt.int16
    Alu = mybir.AluOpType
    Act = mybir.ActivationFunctionType

    x_d = nc.dram_tensor("x", [P, T, F], f32, kind="ExternalInput").ap()
    wgt_d = nc.dram_tensor("wgt", [P, 2 * P], f32, kind="ExternalInput").ap()
    wpk_d = nc.dram_tensor("wpk", [P, WPACK * P], f16, kind="ExternalInput").ap()
    o_d = nc.dram_tensor("out", [P, NW, F], i16, kind="ExternalOutput").ap()

    with tile.TileContext(nc) as tc:
        with (
            tc.tile_pool(name="xp", bufs=6) as xpool,
            tc.tile_pool(name="vp", bufs=3) as vpool,
            tc.tile_pool(name="sg", bufs=4) as sgpool,
            tc.tile_pool(name="wp", bufs=1) as wpool,
            tc.tile_pool(name="mp", bufs=1) as mpool,
            tc.tile_pool(name="op", bufs=2) as opool,
            tc.psum_pool(name="pv", bufs=3) as pvpool,
            tc.psum_pool(name="pw", bufs=2) as pwpool,
            tc.psum_pool(name="pu", bufs=1) as pupool,
        ):
            # weights: [diag(1) | diag(0.25)] f32 and diag(4^j) f16, DMA'd once
            # on the scalar ring so the sync ring starts streaming x at once.
            wgt = wpool.tile([P, 2 * P], f32, name="wgt")
            wpk = wpool.tile([P, WPACK * P], f16, name="wpk")
            nc.sync.dma_start(out=wgt[:], in_=wgt_d)
            nc.scalar.dma_start(out=wpk[:], in_=wpk_d)
            w1 = wgt[:, 0:P]
            w025 = wgt[:, P:2 * P]

            nbias = mpool.tile([P, 1], f32, name="nbias")
            nc.vector.memset(nbias[:], -VTH)
            mem_e = mpool.tile([P, E], f32, name="mem_e")   # PE-chain carry
            nc.vector.memset(mem_e[:], 0.0)
            mem_a = mpool.tile([P, A], f32, name="mem_a")   # DVE-chain carry
            nc.vector.memset(mem_a[:], 0.0)
            memg = mpool.tile([P, W], f32, name="memg")     # gpsimd carry
            nc.gpsimd.memset(memg[:], 0.0)

            xt = [None] * T          # per-block x tiles, indexed by step
            vps = [None] * T         # per-step PSUM v tiles (E cols)
            sgn = [None] * T         # per-step sign tiles
            pword = [None] * NW      # per-window packed-word PSUM tiles
            pending = []             # completed windows awaiting convert
            pending_st = []          # converted windows awaiting store

            def flush_pending():
                # Stage the convert (this step) and the store dma (next step)
                # so the dma's sequencer-held sem wait is already satisfied
                # when it decodes — otherwise it blocks ACT SEQ for ~2us.
                while pending_st:
                    wi, wo = pending_st.pop(0)
                    nc.sync.dma_start(out=o_d[:, wi, :], in_=wo[:])
                while pending:
                    wi = pending.pop(0)
                    wo = opool.tile([P, F], i16, name="wo")
                    nc.scalar.activation(wo[:], pword[wi][:], Act.Copy)
                    pending_st.append((wi, wo))

            def load_block(blk, split=False):
                t0 = blk * TB
                xb = xpool.tile([P, TB, F], f32, name="xb")
                if split:
                    # per-step DMAs so step t0 can start after 1/TB of the
                    # block arrives (pipeline fill)
                    for j in range(TB):
                        nc.sync.dma_start(out=xb[:, j], in_=x_d[:, t0 + j, :])
                else:
                    nc.sync.dma_start(out=xb[:], in_=x_d[:, t0:t0 + TB, :])
                for j in range(TB):
                    xt[t0 + j] = xb

            def mm_x(t):
                # start the v accumulation for step t with the x term
                vps[t] = pvpool.tile([P, E], f32, name="vps")
                nc.tensor.matmul(out=vps[t][:], lhsT=w1, rhs=xt[t][:, t % TB, 0:E],
                                 start=True, stop=False, skip_group_check=True)

            def pack(t):
                # accumulate sign_t * 4^(t%WPACK) into the window word; when
                # the window completes, convert it to int16 and store it.
                wi, j = divmod(t, WPACK)
                if j == 0:
                    pword[wi] = pwpool.tile([P, F], f32, name="pword")
                nc.tensor.matmul(out=pword[wi][:], lhsT=wpk[:, j * P:(j + 1) * P],
                                 rhs=sgn[t][:], start=(j == 0), stop=(j == WPACK - 1),
                                 skip_group_check=True)
                if j == WPACK - 1:
                    pending.append(wi)

            load_block(0, split=True)
            load_block(1, split=True)
            # warm the PE p-state ramp (~3us of tiny matmuls) while the first
            # x block and the weights stream in, so the first real matmuls run
            # at full clock. Warmups use a memset scratch tile so they start
            # immediately; psum slices rotate so WAW never idles the ramp.
            wsc = mpool.tile([P, P], f32, name="wsc")
            nc.vector.memset(wsc[:], 0.0)
            wups = pupool.tile([P, 256], f32, name="wups")
            for i in range(14):
                sl = (i % 4) * 64
                nc.tensor.matmul(out=wups[:, sl:sl + 64], lhsT=wsc[:],
                                 rhs=wsc[:, 0:64], start=True, stop=True,
                                 skip_group_check=True)
            mm_x(0)
            for t in range(T):
                blk, j = divmod(t, TB)
                if j == 0:
                    for nb in (blk + 2, blk + 3):
                        if nb < T // TB and xt[nb * TB] is None:
                            load_block(nb)
                # PE: packs lag 2 steps so sign production never stalls the
                # v-chain; then finish v_t and prefill x for t+1.
                if t >= 2:
                    pack(t - 2)
                # The E-cols' serial loop is PE -> ACT(sign) -> DVE(reset) ->
                # PE; split E into two independently-phased halves so each
                # half's loop latency can span ~2 step periods.
                H = E // 2
                sgn[t] = sgpool.tile([P, F], f16, name="sgn")
                for h0, h1 in ((0, H), (H, E)):
                    nc.tensor.matmul(out=vps[t][:, h0:h1], lhsT=w025,
                                     rhs=mem_e[:, h0:h1], start=False,
                                     stop=True, skip_group_check=True)
                    nc.scalar.activation(sgn[t][:, h0:h1], vps[t][:, h0:h1],
                                         Act.Sign, bias=nbias[:])
                    # E-reset: (sgn<=0)*v == (v<=1)*v (sgn = Sign(v-1), exact
                    # at the v==1 boundary too). sgn comes from SBUF so only
                    # one operand reads PSUM (the ISA limit).
                    nc.vector.scalar_tensor_tensor(
                        out=mem_e[:, h0:h1], in0=sgn[t][:, h0:h1], scalar=0.0,
                        in1=vps[t][:, h0:h1], op0=Alu.is_le, op1=Alu.mult)
                if t + 1 < T:
                    mm_x(t + 1)

                # DVE: A-chain
                vt = vpool.tile([P, A + W], f32, name="vt")
                nc.vector.scalar_tensor_tensor(
                    out=vt[:, 0:A], in0=mem_a[:], scalar=DECAY,
                    in1=xt[t][:, j, E:E + A], op0=Alu.mult, op1=Alu.add)
                nc.vector.scalar_tensor_tensor(
                    out=mem_a[:], in0=vt[:, 0:A], scalar=VTH,
                    in1=vt[:, 0:A], op0=Alu.is_le, op1=Alu.mult)

                # gpsimd W-chain (Pool-legal ops only)
                dg = vpool.tile([P, W], f32, name="dg")
                nc.gpsimd.tensor_scalar(out=dg[:], in0=memg[:], scalar1=DECAY,
                                        scalar2=None, op0=Alu.mult)
                nc.gpsimd.tensor_tensor(out=vt[:, A:], in0=dg[:],
                                        in1=xt[t][:, j, E + A:], op=Alu.add)
                kg = vpool.tile([P, W], f32, name="kg")
                nc.gpsimd.tensor_scalar(out=kg[:], in0=vt[:, A:], scalar1=VTH,
                                        scalar2=None, op0=Alu.is_le)
                nc.gpsimd.tensor_tensor(out=memg[:], in0=kg[:], in1=vt[:, A:],
                                        op=Alu.mult)

                # ACT: signs for the A+W columns (E signs done above)
                nc.scalar.activation(sgn[t][:, E:], vt[:], Act.Sign,
                                     bias=nbias[:])
                flush_pending()

            pack(T - 2)
            pack(T - 1)
            flush_pending()
            flush_pending()
    nc.compile()
    return nc


def _get_nc():
    if "nc" not in _CACHE:
        _CACHE["nc"] = _build_program()
    return _CACHE["nc"]


def _weights():
    w1 = np.eye(P, dtype=np.float32)
    w025 = (0.25 * np.eye(P)).astype(np.float32)
    wgt = np.concatenate([w1, w025], axis=1)                    # [P, 2P]
    wpk = np.concatenate(
        [(float(4 ** j) * np.eye(P)).astype(np.float16) for j in range(WPACK)],
        axis=1)                                                 # [P, 8P]
    return wgt, wpk


def _get_runner():
    """Cache one jitted SPMD executable."""
    if "runner" in _CACHE:
        return _CACHE["runner"]

    import jax
    from jax.sharding import Mesh, PartitionSpec
    from jax.experimental.shard_map import shard_map
    from concourse import bass2jax

    nc = _get_nc()
    bass2jax.install_neuronx_cc_hook()

    in_names = ("x", "wgt", "wpk", "out", "partition_id")
    out_names = ("out",)
    out_avals = (jax.core.ShapedArray((P, NW, F), np.int16),)

    def _body(*args):
        outs = bass2jax._bass_exec_p.bind(
            *args,
            bass2jax.partition_id_tensor(),
            out_avals=out_avals,
            in_names=in_names,
            out_names=out_names,
            lowering_input_output_aliases=(),
            sim_require_finite=True,
            sim_require_nnan=True,
            nc=nc,
        )
        return tuple(outs)

    devices = jax.devices()[:NCORES]
    mesh = Mesh(np.asarray(devices), ("core",))
    sharded = jax.jit(
        shard_map(
            _body,
            mesh=mesh,
            in_specs=(PartitionSpec("core"), PartitionSpec(), PartitionSpec(),
                      PartitionSpec("core")),
            out_specs=(PartitionSpec("core"),),
            check_rep=False,
        ),
        donate_argnums=(3,),
        keep_unused=True,
    )
    _CACHE["runner"] = sharded
    return sharded


def _run_sharded(x_concat):
    """x_concat: [NCORES*P, T, F] host array, core k's slab at rows k*P:(k+1)*P."""
    runner = _get_runner()
    wgt, wpk = _weights()
    zeros = np.zeros((NCORES * P, NW, F), np.int16)
    (out,) = runner(x_concat, wgt, wpk, zeros)
    return np.asarray(out)


def kernel(x):
    x = np.asarray(x, dtype=np.float32)
    assert x.shape == (T, B, N), x.shape
    # [T, B, N] -> [T, 8, P, F] -> per-core [8, P, T, F] -> concat on axis 0
    x_concat = np.ascontiguousarray(
        x.reshape(T, NCORES, P, F).transpose(1, 2, 0, 3)
    ).reshape(NCORES * P, T, F)
    words = _run_sharded(x_concat)                  # [8*P, NW, F] int16
    # decode balanced base-4 digit words: word = sum_j c_j 4^j, c in {-1,0,1};
    # spike at step (wi*WPACK + j) iff c_j == +1.
    w = words.astype(np.int32)                       # [8P, NW, F]
    spikes = np.empty((T, NCORES * P, F), np.float32)
    for j in range(WPACK - 1, -1, -1):
        scale = 4 ** j
        c = np.rint(w * (1.0 / scale)).astype(np.int32)
        np.clip(c, -1, 1, out=c)
        w -= c * scale
        # steps j, WPACK+j, 2*WPACK+j, ... come from digit j of each word
        spikes[j::WPACK] = (c == 1).transpose(1, 0, 2)
    # [T, 8P, F] -> [T, 8, P, F] -> [T, B, N]
    out = spikes.reshape(T, NCORES, P, F).reshape(T, B, N)
    return out


# revision 32
# speedup vs baseline: 1.0722x; 1.0722x over previous
"""Trainium2 Bass kernel for a leaky-integrate-fire (LIF) scan.

Reference computation (forward values only):
    v_t   = mem_{t-1} * 0.25 + x_t          (mem_0 carry = 0)
    s_t   = (v_t > 1.0) ? 1.0 : 0.0         (spike, the output)
    mem_t = (v_t <= 1.0) * v_t              (hard reset)

x: [T=32, B=64, N=16384] f32. Elementwise over (B, N), sequential over T.
Sharding: data-parallel over flattened B*N across 8 cores. Each core's slab
is laid out [P=128, T=32, F=1024] in DRAM.

Design (per core, per step):
- A cols: DVE chain, 2 scalar_tensor_tensor ops (exact f32).
- W cols: gpsimd (Pool) chain with tensor_scalar/tensor_tensor (exact f32).
- ACT computes Sign(v-1) in {-1,0,+1} as fp16 for all F columns (exact:
  v-1 is Sterbenz-exact in [0.5,2), so sign(v-1)==1 <=> v>1 exactly).
- PE packs 8 steps of signs into one PSUM word via diag(4^j) fp16 matmuls
  (word = sum_j sign_j 4^j, a balanced base-4 integer, |word| <= 21845,
  all arithmetic exact), ACT converts each completed word to int16, and the
  host decodes the digits. This cuts store traffic 4x vs int8-per-step and
  leaves the output bit-exact vs the f32 reference.
- PE p-state is pre-warmed with tiny matmuls so pack matmuls run at full
  clock; the first two x blocks load with per-step DMAs for fast pipeline
  fill; stores ride the sync ring so they never stall ACT's sequencer.
"""

import numpy as np

T = 32
B = 64
N = 16384
NCORES = 8
P = 128                      # SBUF partitions
F = (B // NCORES) * N // P   # 1024 free-dim columns per step per core
TB = 4                       # timesteps per DMA block (2 MiB loads)
WPACK = 8                    # steps packed per output word
NW = T // WPACK              # output words per column
A = 812                      # columns on the DVE chain
W = F - A                    # columns on the gpsimd chain
DECAY = 0.25
VTH = 1.0

_CACHE = {}


def _build_program():
    import concourse.bacc as bacc
    import concourse.tile as tile
    from concourse import mybir

    nc = bacc.Bacc(
        target_bir_lowering=False,
        debug=False,
        enable_asserts=False,
        num_devices=NCORES,
    )
    f32 = mybir.dt.float32
    f16 = mybir.dt.float16
    i16 = mybir.dt.int16
    Alu = mybir.AluOpType
    Act = mybir.ActivationFunctionType

    x_d = nc.dram_tensor("x", [P, T, F], f32, kind="ExternalInput").ap()
    wpk_d = nc.dram_tensor("wpk", [P, WPACK * P], f16, kind="ExternalInput").ap()
    o_d = nc.dram_tensor("out", [P, NW, F], i16, kind="ExternalOutput").ap()

    with tile.TileContext(nc) as tc:
        with (
            tc.tile_pool(name="xp", bufs=6) as xpool,
            tc.tile_pool(name="vp", bufs=3) as vpool,
            tc.tile_pool(name="sg", bufs=4) as sgpool,
            tc.tile_pool(name="wp", bufs=1) as wpool,
            tc.tile_pool(name="mp", bufs=1) as mpool,
            tc.tile_pool(name="op", bufs=2) as opool,
            tc.psum_pool(name="pw", bufs=2) as pwpool,
            tc.psum_pool(name="pu", bufs=1) as pupool,
        ):
            # pack weights diag(4^j) f16, DMA'd once
            wpk = wpool.tile([P, WPACK * P], f16, name="wpk")
            nc.scalar.dma_start(out=wpk[:], in_=wpk_d)

            nbias = mpool.tile([P, 1], f32, name="nbias")
            nc.vector.memset(nbias[:], -VTH)
            mem_a = mpool.tile([P, A], f32, name="mem_a")   # DVE-chain carry
            nc.vector.memset(mem_a[:], 0.0)
            memg = mpool.tile([P, W], f32, name="memg")     # gpsimd carry
            nc.gpsimd.memset(memg[:], 0.0)

            # warm the PE p-state ramp with tiny matmuls on a memset scratch
            # tile so pack matmuls run at full clock from the start
            wsc = mpool.tile([P, P], f32, name="wsc")
            nc.vector.memset(wsc[:], 0.0)
            wups = pupool.tile([P, 256], f32, name="wups")
            for i in range(14):
                sl = (i % 4) * 64
                nc.tensor.matmul(out=wups[:, sl:sl + 64], lhsT=wsc[:],
                                 rhs=wsc[:, 0:64], start=True, stop=True,
                                 skip_group_check=True)

            xt = [None] * T          # per-block x tiles, indexed by step
            sgn = [None] * T         # per-step sign tiles
            pword = [None] * NW      # per-window packed-word PSUM tiles
            pending = []             # completed windows awaiting convert
            pending_st = []          # converted windows awaiting store

            def flush_pending():
                # convert this step; store next step (so the store dma's
                # sequencer-held sem wait is already satisfied at decode)
                while pending_st:
                    wi, wo = pending_st.pop(0)
                    nc.sync.dma_start(out=o_d[:, wi, :], in_=wo[:])
                while pending:
                    wi = pending.pop(0)
                    wo = opool.tile([P, F], i16, name="wo")
                    nc.scalar.activation(wo[:], pword[wi][:], Act.Copy)
                    pending_st.append((wi, wo))

            def load_block(blk, split=False):
                t0 = blk * TB
                xb = xpool.tile([P, TB, F], f32, name="xb")
                if split:
                    for j in range(TB):
                        nc.sync.dma_start(out=xb[:, j], in_=x_d[:, t0 + j, :])
                else:
                    nc.sync.dma_start(out=xb[:], in_=x_d[:, t0:t0 + TB, :])
                for j in range(TB):
                    xt[t0 + j] = xb

            def pack(t):
                # accumulate sign_t * 4^(t%WPACK) into the window word
                wi, j = divmod(t, WPACK)
                if j == 0:
                    pword[wi] = pwpool.tile([P, F], f32, name="pword")
                nc.tensor.matmul(out=pword[wi][:], lhsT=wpk[:, j * P:(j + 1) * P],
                                 rhs=sgn[t][:], start=(j == 0), stop=(j == WPACK - 1),
                                 skip_group_check=True)
                if j == WPACK - 1:
                    pending.append(wi)

            load_block(0, split=True)
            load_block(1, split=True)
            for t in range(T):
                blk, j = divmod(t, TB)
                if j == 0:
                    for nb in (blk + 2, blk + 3):
                        if nb < T // TB and xt[nb * TB] is None:
                            load_block(nb)
                # PE: packs lag 2 steps so sign production never stalls them
                if t >= 2:
                    pack(t - 2)

                # DVE A-chain
                vt = vpool.tile([P, F], f32, name="vt")
                nc.vector.scalar_tensor_tensor(
                    out=vt[:, 0:A], in0=mem_a[:], scalar=DECAY,
                    in1=xt[t][:, j, 0:A], op0=Alu.mult, op1=Alu.add)
                nc.vector.scalar_tensor_tensor(
                    out=mem_a[:], in0=vt[:, 0:A], scalar=VTH,
                    in1=vt[:, 0:A], op0=Alu.is_le, op1=Alu.mult)

                # gpsimd W-chain (Pool-legal ops only)
                dg = vpool.tile([P, W], f32, name="dg")
                nc.gpsimd.tensor_scalar(out=dg[:], in0=memg[:], scalar1=DECAY,
                                        scalar2=None, op0=Alu.mult)
                nc.gpsimd.tensor_tensor(out=vt[:, A:], in0=dg[:],
                                        in1=xt[t][:, j, A:], op=Alu.add)
                kg = vpool.tile([P, W], f32, name="kg")
                nc.gpsimd.tensor_scalar(out=kg[:], in0=vt[:, A:], scalar1=VTH,
                                        scalar2=None, op0=Alu.is_le)
                nc.gpsimd.tensor_tensor(out=memg[:], in0=kg[:], in1=vt[:, A:],
                                        op=Alu.mult)

                # ACT: signs for all F columns in one pass
                sgn[t] = sgpool.tile([P, F], f16, name="sgn")
                nc.scalar.activation(sgn[t][:], vt[:], Act.Sign, bias=nbias[:])
                flush_pending()

            pack(T - 2)
            pack(T - 1)
            flush_pending()
            flush_pending()
    nc.compile()
    return nc


def _get_nc():
    if "nc" not in _CACHE:
        _CACHE["nc"] = _build_program()
    return _CACHE["nc"]


def _weights():
    wpk = np.concatenate(
        [(float(4 ** j) * np.eye(P)).astype(np.float16) for j in range(WPACK)],
        axis=1)                                                 # [P, 8P]
    return wpk


def _get_runner():
    """Cache one jitted SPMD executable."""
    if "runner" in _CACHE:
        return _CACHE["runner"]

    import jax
    from jax.sharding import Mesh, PartitionSpec
    from jax.experimental.shard_map import shard_map
    from concourse import bass2jax

    nc = _get_nc()
    bass2jax.install_neuronx_cc_hook()

    in_names = ("x", "wpk", "out", "partition_id")
    out_names = ("out",)
    out_avals = (jax.core.ShapedArray((P, NW, F), np.int16),)

    def _body(*args):
        outs = bass2jax._bass_exec_p.bind(
            *args,
            bass2jax.partition_id_tensor(),
            out_avals=out_avals,
            in_names=in_names,
            out_names=out_names,
            lowering_input_output_aliases=(),
            sim_require_finite=True,
            sim_require_nnan=True,
            nc=nc,
        )
        return tuple(outs)

    devices = jax.devices()[:NCORES]
    mesh = Mesh(np.asarray(devices), ("core",))
    sharded = jax.jit(
        shard_map(
            _body,
            mesh=mesh,
            in_specs=(PartitionSpec("core"), PartitionSpec(),
                      PartitionSpec("core")),
            out_specs=(PartitionSpec("core"),),
            check_rep=False,
        ),
        donate_argnums=(2,),
        keep_unused=True,
    )
    _CACHE["runner"] = sharded
    return sharded


def _run_sharded(x_concat):
    """x_concat: [NCORES*P, T, F] host array, core k's slab at rows k*P:(k+1)*P."""
    runner = _get_runner()
    wpk = _weights()
    zeros = np.zeros((NCORES * P, NW, F), np.int16)
    (out,) = runner(x_concat, wpk, zeros)
    return np.asarray(out)


def kernel(x):
    x = np.asarray(x, dtype=np.float32)
    assert x.shape == (T, B, N), x.shape
    # [T, B, N] -> [T, 8, P, F] -> per-core [8, P, T, F] -> concat on axis 0
    x_concat = np.ascontiguousarray(
        x.reshape(T, NCORES, P, F).transpose(1, 2, 0, 3)
    ).reshape(NCORES * P, T, F)
    words = _run_sharded(x_concat)                  # [8*P, NW, F] int16
    # decode balanced base-4 digit words: word = sum_j c_j 4^j, c in {-1,0,1};
    # spike at step (wi*WPACK + j) iff c_j == +1.
    w = words.astype(np.int32)                       # [8P, NW, F]
    spikes = np.empty((T, NCORES * P, F), np.float32)
    for j in range(WPACK - 1, -1, -1):
        scale = 4 ** j
        c = np.rint(w * (1.0 / scale)).astype(np.int32)
        np.clip(c, -1, 1, out=c)
        w -= c * scale
        spikes[j::WPACK] = (c == 1).transpose(1, 0, 2)
    # [T, 8P, F] -> [T, 8, P, F] -> [T, B, N]
    out = spikes.reshape(T, NCORES, P, F).reshape(T, B, N)
    return out


# revision 36
# speedup vs baseline: 1.1270x; 1.0512x over previous
"""Trainium2 Bass kernel for a leaky-integrate-fire (LIF) scan.

Reference computation (forward values only):
    v_t   = mem_{t-1} * 0.25 + x_t          (mem_0 carry = 0)
    s_t   = (v_t > 1.0) ? 1.0 : 0.0         (spike, the output)
    mem_t = (v_t <= 1.0) * v_t              (hard reset)

x: [T=32, B=64, N=16384] f32. Elementwise over (B, N), sequential over T.
Sharding: data-parallel over flattened B*N across 8 cores. Each core's slab
is laid out [P=128, T=32, F=1024] in DRAM.

Design (per core, per step):
- A cols: DVE chain, 2 scalar_tensor_tensor ops (exact f32).
- W cols: gpsimd (Pool) chain with tensor_scalar/tensor_tensor (exact f32).
- ACT computes Sign(v-1) in {-1,0,+1} as fp16 for all F columns (exact:
  v-1 is Sterbenz-exact in [0.5,2), so sign(v-1)==1 <=> v>1 exactly).
- PE packs 8 steps of signs into one PSUM word via diag(4^j) fp16 matmuls
  (word = sum_j sign_j 4^j, a balanced base-4 integer, |word| <= 21845,
  all arithmetic exact), ACT converts each completed word to int16, and the
  host decodes the digits. This cuts store traffic 4x vs int8-per-step and
  leaves the output bit-exact vs the f32 reference.
- PE p-state is pre-warmed with tiny matmuls so pack matmuls run at full
  clock; the first two x blocks load with per-step DMAs for fast pipeline
  fill; stores ride the sync ring so they never stall ACT's sequencer.
"""

import numpy as np

T = 32
B = 64
N = 16384
NCORES = 8
P = 128                      # SBUF partitions
F = (B // NCORES) * N // P   # 1024 free-dim columns per step per core
TB = 4                       # timesteps per DMA block (2 MiB loads)
WPACK = 8                    # steps packed per output word
NW = T // WPACK              # output words per column
A = 828                      # columns on the DVE chain (two halves)
W = F - A                    # columns on the gpsimd chain
A1 = A // 2
DECAY = 0.25
VTH = 1.0

_CACHE = {}


def _build_program():
    import concourse.bacc as bacc
    import concourse.tile as tile
    from concourse import mybir

    nc = bacc.Bacc(
        target_bir_lowering=False,
        debug=False,
        enable_asserts=False,
        num_devices=NCORES,
    )
    f32 = mybir.dt.float32
    f16 = mybir.dt.float16
    i16 = mybir.dt.int16
    Alu = mybir.AluOpType
    Act = mybir.ActivationFunctionType

    x_d = nc.dram_tensor("x", [P, T, F], f32, kind="ExternalInput").ap()
    wpk_d = nc.dram_tensor("wpk", [P, WPACK * P], f16, kind="ExternalInput").ap()
    o_d = nc.dram_tensor("out", [P, NW, F], i16, kind="ExternalOutput").ap()

    with tile.TileContext(nc) as tc:
        with (
            tc.tile_pool(name="xp", bufs=6) as xpool,
            tc.tile_pool(name="vp", bufs=5) as vpool,
            tc.tile_pool(name="sg", bufs=6) as sgpool,
            tc.tile_pool(name="wp", bufs=1) as wpool,
            tc.tile_pool(name="mp", bufs=1) as mpool,
            tc.tile_pool(name="op", bufs=2) as opool,
            tc.psum_pool(name="pw", bufs=2) as pwpool,
            tc.psum_pool(name="pu", bufs=1) as pupool,
        ):
            # pack weights diag(4^j) f16, DMA'd once
            wpk = wpool.tile([P, WPACK * P], f16, name="wpk")
            nc.scalar.dma_start(out=wpk[:], in_=wpk_d)

            nbias = mpool.tile([P, 1], f32, name="nbias")
            nc.vector.memset(nbias[:], -VTH)
            # two independent DVE-chain carries: interleaving the two halves'
            # ops hides the same-engine RAW write-ack bubble between the
            # dependent STT pairs
            mem_a1 = mpool.tile([P, A1], f32, name="mem_a1")
            nc.vector.memset(mem_a1[:], 0.0)
            mem_a2 = mpool.tile([P, A - A1], f32, name="mem_a2")
            nc.vector.memset(mem_a2[:], 0.0)
            memg = mpool.tile([P, W], f32, name="memg")     # gpsimd carry
            nc.gpsimd.memset(memg[:], 0.0)

            # warm the PE p-state ramp with tiny matmuls on a memset scratch
            # tile so pack matmuls run at full clock from the start
            wsc = mpool.tile([P, P], f32, name="wsc")
            nc.vector.memset(wsc[:], 0.0)
            wups = pupool.tile([P, 256], f32, name="wups")
            for i in range(14):
                sl = (i % 4) * 64
                nc.tensor.matmul(out=wups[:, sl:sl + 64], lhsT=wsc[:],
                                 rhs=wsc[:, 0:64], start=True, stop=True,
                                 skip_group_check=True)

            xt = [None] * T          # per-block x tiles, indexed by step
            sgn = [None] * T         # per-step sign tiles
            pword = [None] * NW      # per-window packed-word PSUM tiles
            pending = []             # completed windows awaiting convert
            pending_st = []          # converted windows awaiting store

            def flush_pending():
                # convert this step; store next step (so the store dma's
                # sequencer-held sem wait is already satisfied at decode)
                while pending_st:
                    wi, wo = pending_st.pop(0)
                    nc.sync.dma_start(out=o_d[:, wi, :], in_=wo[:])
                while pending:
                    wi = pending.pop(0)
                    wo = opool.tile([P, F], i16, name="wo")
                    nc.scalar.activation(wo[:], pword[wi][:], Act.Copy)
                    pending_st.append((wi, wo))

            def load_block(blk, split=False):
                t0 = blk * TB
                xb = xpool.tile([P, TB, F], f32, name="xb")
                if split:
                    for j in range(TB):
                        nc.sync.dma_start(out=xb[:, j], in_=x_d[:, t0 + j, :])
                else:
                    nc.sync.dma_start(out=xb[:], in_=x_d[:, t0:t0 + TB, :])
                for j in range(TB):
                    xt[t0 + j] = xb

            def pack(t):
                # accumulate sign_t * 4^(t%WPACK) into the window word
                wi, j = divmod(t, WPACK)
                if j == 0:
                    pword[wi] = pwpool.tile([P, F], f32, name="pword")
                nc.tensor.matmul(out=pword[wi][:], lhsT=wpk[:, j * P:(j + 1) * P],
                                 rhs=sgn[t][:], start=(j == 0), stop=(j == WPACK - 1),
                                 skip_group_check=True)
                if j == WPACK - 1:
                    pending.append(wi)

            load_block(0, split=True)
            load_block(1, split=True)
            for t in range(T):
                blk, j = divmod(t, TB)
                if j == 0:
                    for nb in (blk + 2, blk + 3):
                        if nb < T // TB and xt[nb * TB] is None:
                            load_block(nb)
                # PE: packs lag 2 steps so sign production never stalls them
                if t >= 2:
                    pack(t - 2)

                # DVE A-chain, two interleaved halves: v1, v2, r1, r2
                vt = vpool.tile([P, F], f32, name="vt")
                nc.vector.scalar_tensor_tensor(
                    out=vt[:, 0:A1], in0=mem_a1[:], scalar=DECAY,
                    in1=xt[t][:, j, 0:A1], op0=Alu.mult, op1=Alu.add)
                nc.vector.scalar_tensor_tensor(
                    out=vt[:, A1:A], in0=mem_a2[:], scalar=DECAY,
                    in1=xt[t][:, j, A1:A], op0=Alu.mult, op1=Alu.add)
                nc.vector.scalar_tensor_tensor(
                    out=mem_a1[:], in0=vt[:, 0:A1], scalar=VTH,
                    in1=vt[:, 0:A1], op0=Alu.is_le, op1=Alu.mult)
                nc.vector.scalar_tensor_tensor(
                    out=mem_a2[:], in0=vt[:, A1:A], scalar=VTH,
                    in1=vt[:, A1:A], op0=Alu.is_le, op1=Alu.mult)

                # gpsimd W-chain (Pool-legal ops only)
                dg = vpool.tile([P, W], f32, name="dg")
                nc.gpsimd.tensor_scalar(out=dg[:], in0=memg[:], scalar1=DECAY,
                                        scalar2=None, op0=Alu.mult)
                nc.gpsimd.tensor_tensor(out=vt[:, A:], in0=dg[:],
                                        in1=xt[t][:, j, A:], op=Alu.add)
                kg = vpool.tile([P, W], f32, name="kg")
                nc.gpsimd.tensor_scalar(out=kg[:], in0=vt[:, A:], scalar1=VTH,
                                        scalar2=None, op0=Alu.is_le)
                nc.gpsimd.tensor_tensor(out=memg[:], in0=kg[:], in1=vt[:, A:],
                                        op=Alu.mult)

                # ACT: signs for all F columns in one pass
                sgn[t] = sgpool.tile([P, F], f16, name="sgn")
                nc.scalar.activation(sgn[t][:], vt[:], Act.Sign, bias=nbias[:])
                flush_pending()

            pack(T - 2)
            pack(T - 1)
            flush_pending()
            flush_pending()
    nc.compile()
    return nc


def _get_nc():
    if "nc" not in _CACHE:
        _CACHE["nc"] = _build_program()
    return _CACHE["nc"]


def _weights():
    wpk = np.concatenate(
        [(float(4 ** j) * np.eye(P)).astype(np.float16) for j in range(WPACK)],
        axis=1)                                                 # [P, 8P]
    return wpk


def _get_runner():
    """Cache one jitted SPMD executable."""
    if "runner" in _CACHE:
        return _CACHE["runner"]

    import jax
    from jax.sharding import Mesh, PartitionSpec
    from jax.experimental.shard_map import shard_map
    from concourse import bass2jax

    nc = _get_nc()
    bass2jax.install_neuronx_cc_hook()

    in_names = ("x", "wpk", "out", "partition_id")
    out_names = ("out",)
    out_avals = (jax.core.ShapedArray((P, NW, F), np.int16),)

    def _body(*args):
        outs = bass2jax._bass_exec_p.bind(
            *args,
            bass2jax.partition_id_tensor(),
            out_avals=out_avals,
            in_names=in_names,
            out_names=out_names,
            lowering_input_output_aliases=(),
            sim_require_finite=True,
            sim_require_nnan=True,
            nc=nc,
        )
        return tuple(outs)

    devices = jax.devices()[:NCORES]
    mesh = Mesh(np.asarray(devices), ("core",))
    sharded = jax.jit(
        shard_map(
            _body,
            mesh=mesh,
            in_specs=(PartitionSpec("core"), PartitionSpec(),
                      PartitionSpec("core")),
            out_specs=(PartitionSpec("core"),),
            check_rep=False,
        ),
        donate_argnums=(2,),
        keep_unused=True,
    )
    _CACHE["runner"] = sharded
    return sharded


def _run_sharded(x_concat):
    """x_concat: [NCORES*P, T, F] host array, core k's slab at rows k*P:(k+1)*P."""
    runner = _get_runner()
    wpk = _weights()
    zeros = np.zeros((NCORES * P, NW, F), np.int16)
    (out,) = runner(x_concat, wpk, zeros)
    return np.asarray(out)


def kernel(x):
    x = np.asarray(x, dtype=np.float32)
    assert x.shape == (T, B, N), x.shape
    # [T, B, N] -> [T, 8, P, F] -> per-core [8, P, T, F] -> concat on axis 0
    x_concat = np.ascontiguousarray(
        x.reshape(T, NCORES, P, F).transpose(1, 2, 0, 3)
    ).reshape(NCORES * P, T, F)
    words = _run_sharded(x_concat)                  # [8*P, NW, F] int16
    # decode balanced base-4 digit words: word = sum_j c_j 4^j, c in {-1,0,1};
    # spike at step (wi*WPACK + j) iff c_j == +1.
    w = words.astype(np.int32)                       # [8P, NW, F]
    spikes = np.empty((T, NCORES * P, F), np.float32)
    for j in range(WPACK - 1, -1, -1):
        scale = 4 ** j
        c = np.rint(w * (1.0 / scale)).astype(np.int32)
        np.clip(c, -1, 1, out=c)
        w -= c * scale
        spikes[j::WPACK] = (c == 1).transpose(1, 0, 2)
    # [T, 8P, F] -> [T, 8, P, F] -> [T, B, N]
    out = spikes.reshape(T, NCORES, P, F).reshape(T, B, N)
    return out
